# revision 1
# baseline (speedup 1.0000x reference)
"""Bahdanau-attention Bass/Tile kernel for TRN2.

Computation (per core, data-parallel over batch):
  proj_h = h @ Wh.T + b_attn                    [B, H]
  energy = tanh(enc @ We.T + proj_h[:, None])   [B, S, H]
  scores = energy @ v                           [B, S]
  attn   = softmax(scores, axis=S)              [B, S]
  ctx    = attn @ enc                           [B, H]

Layout strategy: energy tiles are [o=128, s=512] so proj_h is a per-partition
ACT bias fused into tanh, and the v-dot is a PE matmul with v as the
stationary column.  The big matmul runs in float32r (TF32-like, 1 cyc/row at
N>=512).  enc is transposed on-device by the PE (fp32 DMA transpose is
unsupported) and read from HBM exactly once: ctx is accumulated per s-chunk
from the transposed tiles on DVE with online-softmax rescaling, so no state
outlives a chunk except the [128, KO] accumulator.
"""

from contextlib import ExitStack

import numpy as np

import concourse.bass as bass
import concourse.tile as tile
import concourse.mybir as mybir
from concourse import bacc
from concourse.masks import make_identity

F32 = mybir.dt.float32
F32R = mybir.dt.float32r
AFT = mybir.ActivationFunctionType
AX = mybir.AxisListType
ALU = mybir.AluOpType


def build(B_pc=4, S=2048, H=1024, n_loop=1, n_cores=8):
    P = 128
    KO = H // P          # k-subtiles over h
    OT = H // P          # o-tiles
    SCH = 512            # s-chunk size
    SC = S // SCH        # s-chunks per batch
    CT = SCH // P        # 128-blocks per s-chunk

    nc = bacc.Bacc("TRN2", target_bir_lowering=False, debug=False,
                   num_devices=n_cores)

    enc = nc.dram_tensor("enc", [B_pc, S, H], F32R, kind="ExternalInput").ap()
    weT = nc.dram_tensor("weT", [H, H], F32R, kind="ExternalInput").ap()
    whT = nc.dram_tensor("whT", [H, H], F32, kind="ExternalInput").ap()
    hT = nc.dram_tensor("hT", [H, B_pc], F32, kind="ExternalInput").ap()
    bias = nc.dram_tensor("bias", [H], F32, kind="ExternalInput").ap()
    v = nc.dram_tensor("v", [H], F32R, kind="ExternalInput").ap()
    ctx_out = nc.dram_tensor("ctx", [B_pc, H], F32, kind="ExternalOutput").ap()
    attn_out = nc.dram_tensor("attn", [B_pc, S], F32, kind="ExternalOutput").ap()

    with tile.TileContext(nc) as tc, ExitStack() as stack:
        singles = stack.enter_context(tc.tile_pool(name="singles", bufs=1))
        enc_pool = stack.enter_context(tc.tile_pool(name="enc", bufs=3))

        def load_enc(b, sc):
            t = enc_pool.tile([P, CT, H], F32R, tag="enc", name="enc")
            nc.gpsimd.dma_start(
                out=t,
                in_=enc[b, sc * SCH:(sc + 1) * SCH, :].rearrange(
                    "(c p) h -> p c h", p=P),
            )
            return t

        # identity + first enc chunks first so PE transposes start immediately
        ident = singles.tile([P, P], F32R)
        ident_f32 = singles.tile([P, P], F32)
        make_identity(nc, ident_f32)
        nc.vector.tensor_copy(out=ident, in_=ident_f32)
        prefetch = {}
        if n_loop == 1:
            prefetch[(0, 0)] = load_enc(0, 0)
            if SC > 1:
                prefetch[(0, 1)] = load_enc(0, 1)

        # --- constants / weights resident in SBUF ---
        weT_sb = singles.tile([P, KO, H], F32R)
        for q in range(2):
            nc.sync.dma_start(
                out=weT_sb[:, q * (KO // 2):(q + 1) * (KO // 2), :],
                in_=weT[q * (H // 2):(q + 1) * (H // 2), :].rearrange(
                    "(ko p) o -> p ko o", p=P))
        v_sb = singles.tile([P, OT], F32R)
        nc.sync.dma_start(out=v_sb, in_=v.rearrange("(ot p) -> p ot", p=P))
        projhT_sb = singles.tile([P, OT, B_pc], F32)

        # encT / transpose staging
        encT_pool = stack.enter_context(tc.tile_pool(name="encT", bufs=4))
        tp_ps = stack.enter_context(tc.tile_pool(name="tp_ps", bufs=3, space="PSUM"))

        pending_dve = []

        def transpose_chunk(enc_nat):
            encT = encT_pool.tile([P, KO, SCH], F32R, tag="encT", name="encT")
            for ko in range(KO):
                tp = tp_ps.tile([P, SCH], F32R, tag="tp", name="tp")
                for c in range(CT):
                    nc.tensor.transpose(
                        tp[:, c * P:(c + 1) * P],
                        enc_nat[:, c, ko * P:(ko + 1) * P],
                        ident,
                    )
                nc.vector.tensor_copy(out=encT[:, ko, :], in_=tp)
                # drain one deferred ctx-reduce op per copy so the copies
                # (which gate the PE) are never starved behind a reduce burst
                if pending_dve:
                    pending_dve.pop(0)()
            return encT

        pre_encT = {}
        if n_loop == 1:
            pre_encT[(0, 0)] = transpose_chunk(prefetch.pop((0, 0)))

        # --- proj_hT[o, b] = sum_h whT[h, o] * hT[h, b] + bias[o] ---
        with tc.tile_pool(name="startup", bufs=1) as startup, \
             tc.tile_pool(name="startup_ps", bufs=2, space="PSUM") as startup_ps:
            whT_sb = startup.tile([P, KO, H], F32)
            for q in range(2):
                nc.sync.dma_start(
                    out=whT_sb[:, q * (KO // 2):(q + 1) * (KO // 2), :],
                    in_=whT[q * (H // 2):(q + 1) * (H // 2), :].rearrange(
                        "(ko p) o -> p ko o", p=P))
            hT_sb = startup.tile([P, KO, B_pc], F32)
            nc.sync.dma_start(out=hT_sb,
                              in_=hT.rearrange("(ko p) b -> p ko b", p=P))
            b_sb = startup.tile([P, OT], F32)
            nc.sync.dma_start(out=b_sb, in_=bias.rearrange("(ot p) -> p ot", p=P))
            for ot in range(OT):
                ph_ps = startup_ps.tile([P, B_pc], F32)
                for ko in range(KO):
                    nc.tensor.matmul(
                        ph_ps,
                        lhsT=whT_sb[:, ko, ot * P:(ot + 1) * P],
                        rhs=hT_sb[:, ko, :],
                        start=(ko == 0), stop=(ko == KO - 1),
                    )
                nc.vector.tensor_add(
                    out=projhT_sb[:, ot, :],
                    in0=ph_ps,
                    in1=b_sb[:, ot:ot + 1].to_broadcast([P, B_pc]),
                )

        # --- main pools ---
        tanh_pool = stack.enter_context(tc.tile_pool(name="tanh", bufs=8))
        scratch_pool = stack.enter_context(tc.tile_pool(name="scratch", bufs=2))
        row_pool = stack.enter_context(tc.tile_pool(name="rows", bufs=2))
        rep_pool = stack.enter_context(tc.tile_pool(name="rep", bufs=2))
        small_pool = stack.enter_context(tc.tile_pool(name="small", bufs=2))
        ctxrow_pool = stack.enter_context(tc.tile_pool(name="ctxrow", bufs=2))
        en_ps = stack.enter_context(tc.tile_pool(name="en_ps", bufs=2, space="PSUM"))
        sc_ps = stack.enter_context(tc.tile_pool(name="sc_ps", bufs=2, space="PSUM"))
        ctp_ps = stack.enter_context(tc.tile_pool(name="ctp_ps", bufs=1, space="PSUM"))

        def sm(tag):
            return small_pool.tile([1, 1], F32, tag=tag, name=tag)

        def body(_i=None):
            for b in range(B_pc):
                attn_sb = row_pool.tile([1, S], F32, tag="attn", name="attn")
                mvec = small_pool.tile([1, SC], F32, tag="mvec", name="mvec")
                m_run = l_run = None
                accs = [small_pool.tile([P, KO], F32, tag="accA", name="accA"),
                        small_pool.tile([P, KO], F32, tag="accB", name="accB")]
                for sc in range(SC):
                    # ---- pass 1: transpose + energy + tanh + v-dot ----
                    encT = pre_encT.pop((b, sc), None)
                    if encT is None:
                        enc_nat = prefetch.pop((b, sc), None)
                        if enc_nat is None:
                            enc_nat = load_enc(b, sc)
                        encT = transpose_chunk(enc_nat)
                    if sc == 0 and b + 1 < B_pc:
                        prefetch[(b + 1, 0)] = load_enc(b + 1, 0)
                    scp = sc_ps.tile([1, SCH], F32, tag="scp", name="scp")
                    ths = []
                    for ot in range(OT):
                        enp = en_ps.tile([P, SCH], F32, tag="enp", name="enp")
                        for ko in range(KO):
                            nc.tensor.matmul(
                                enp,
                                lhsT=weT_sb[:, ko, ot * P:(ot + 1) * P],
                                rhs=encT[:, ko, :],
                                start=(ko == 0), stop=(ko == KO - 1),
                            )
                        th = tanh_pool.tile([P, SCH], F32R, tag="tanh",
                                            name="tanh")
                        nc.scalar.activation(
                            out=th, in_=enp, func=AFT.Tanh,
                            bias=projhT_sb[:, ot, b:b + 1], scale=1.0,
                        )
                        ths.append(th)
                    # v-dots after all energy groups: the PE never stalls on
                    # the ACT tanh round-trip
                    for ot in range(OT):
                        nc.tensor.matmul(
                            scp,
                            lhsT=v_sb[:, ot:ot + 1],
                            rhs=ths[ot],
                            start=(ot == 0), stop=(ot == OT - 1),
                            skip_group_check=True,
                        )

                    # ---- online softmax + ctx accumulation for this chunk ----
                    srow = small_pool.tile([1, SCH], F32, tag="srow", name="srow")
                    nc.vector.tensor_copy(out=srow, in_=scp)
                    m_c = sm("m_c")
                    nc.vector.reduce_max(out=m_c, in_=srow, axis=AX.X)
                    m_new = sm(f"mnew{sc % 2}")
                    if sc == 0:
                        nc.vector.tensor_copy(out=m_new, in_=m_c)
                    else:
                        nc.vector.tensor_tensor(out=m_new, in0=m_run, in1=m_c,
                                                op=ALU.max)
                    nc.vector.tensor_copy(out=mvec[:, sc:sc + 1], in_=m_new)
                    negm = sm("negm")
                    nc.vector.tensor_scalar_mul(negm, m_new, -1.0)
                    p_c = attn_sb[:, sc * SCH:(sc + 1) * SCH]
                    nc.scalar.activation(out=p_c, in_=srow, func=AFT.Exp,
                                         bias=negm, scale=1.0)
                    lsum_c = sm("lsum_c")
                    nc.vector.reduce_sum(out=lsum_c, in_=p_c, axis=AX.X)
                    l_new = sm(f"lnew{sc % 2}")
                    if sc == 0:
                        nc.vector.tensor_copy(out=l_new, in_=lsum_c)
                    else:
                        alpha = sm("alpha")
                        nc.scalar.activation(out=alpha, in_=m_run, func=AFT.Exp,
                                             bias=negm, scale=1.0)
                        nc.vector.tensor_scalar(
                            out=l_new, in0=l_run, scalar1=alpha,
                            scalar2=lsum_c, op0=ALU.mult, op1=ALU.add)
                        alpha_bc = rep_pool.tile([P, 1], F32, tag="alpha_bc",
                                                 name="alpha_bc")
                        nc.gpsimd.partition_broadcast(alpha_bc, alpha)
                    m_run, l_run = m_new, l_new

                    w_b = rep_pool.tile([P, SCH], F32, tag="w_b", name="w_b")
                    nc.gpsimd.partition_broadcast(w_b, p_c)
                    parts = small_pool.tile([P, KO], F32, tag="parts",
                                            name="parts")

                    def red_pair(ko, encT=encT, w_b=w_b, parts=parts):
                        prod = scratch_pool.tile([P, SCH], F32, tag="prod",
                                                 name="prod")
                        nc.vector.tensor_mul(
                            out=prod, in0=encT[:, ko, :].bitcast(F32), in1=w_b)
                        nc.vector.reduce_sum(out=parts[:, ko:ko + 1], in_=prod,
                                             axis=AX.X)

                    def acc_update(sc=sc, parts=parts,
                                   alpha_bc=(None if sc == 0 else alpha_bc)):
                        cur, prev = accs[sc % 2], accs[(sc + 1) % 2]
                        if sc == 0:
                            nc.vector.tensor_copy(out=cur, in_=parts)
                        else:
                            nc.vector.tensor_scalar_mul(cur, prev, alpha_bc)
                            nc.vector.tensor_add(out=cur, in0=cur, in1=parts)

                    for ko in range(KO):
                        pending_dve.append(
                            lambda ko=ko, f=red_pair: f(ko))
                    pending_dve.append(acc_update)

                # ---- batch epilogue ----
                while pending_dve:
                    pending_dve.pop(0)()
                rinv = sm("rinv")
                nc.vector.reciprocal(out=rinv, in_=l_run)
                # attn output: correct each chunk by exp(m_sc - m_fin) / l
                neg_mfin = sm("neg_mfin")
                nc.vector.tensor_scalar_mul(neg_mfin, m_run, -1.0)
                corr = small_pool.tile([1, SC], F32, tag="corr", name="corr")
                nc.scalar.activation(out=corr, in_=mvec, func=AFT.Exp,
                                     bias=neg_mfin, scale=1.0)
                nc.vector.tensor_scalar_mul(corr, corr, rinv)
                for sc in range(SC):
                    nc.vector.tensor_scalar_mul(
                        attn_sb[:, sc * SCH:(sc + 1) * SCH],
                        attn_sb[:, sc * SCH:(sc + 1) * SCH],
                        corr[:, sc:sc + 1])
                nc.sync.dma_start(out=attn_out[b:b + 1, :], in_=attn_sb)
                # ctx output: acc * rinv, transposed to a row
                rinv_bc = rep_pool.tile([P, 1], F32, tag="rinv_bc",
                                        name="rinv_bc")
                nc.gpsimd.partition_broadcast(rinv_bc, rinv)
                final = accs[(SC - 1) % 2]
                ctp = ctp_ps.tile([KO, P], F32, tag="ctp", name="ctp")
                nc.tensor.transpose(ctp, final, ident_f32)
                crow = ctxrow_pool.tile([KO, P], F32, tag="crow", name="crow")
                nc.vector.tensor_scalar_mul(crow, ctp, rinv_bc[0:KO, :])
                nc.sync.dma_start(
                    out=ctx_out[b:b + 1, :].rearrange(
                        "one (ko x) -> (one ko) x", ko=KO),
                    in_=crow)

        if n_loop == 1:
            body()
        else:
            with tc.For_i(0, n_loop, 1) as i:
                body(i)

    nc.compile()
    return nc


_CACHE = {}


def get_nc(**kw):
    key = tuple(sorted(kw.items()))
    if key not in _CACHE:
        _CACHE[key] = build(**kw)
    return _CACHE[key]


def host_prep(hidden, encoder_outputs, W_attn, b_attn, v, n_cores=8):
    """Split full inputs into per-core in_maps."""
    B = encoder_outputs.shape[0]
    B_pc = B // n_cores
    H = W_attn.shape[0]
    h = np.ascontiguousarray(hidden[-1])            # [B, H]
    whT = np.ascontiguousarray(W_attn[:, :H].T)     # [h, o]
    weT = np.ascontiguousarray(W_attn[:, H:].T)     # [h, o]
    b_attn = np.ascontiguousarray(b_attn)
    v = np.ascontiguousarray(v)
    in_maps = []
    for c in range(n_cores):
        sl = slice(c * B_pc, (c + 1) * B_pc)
        in_maps.append({
            "enc": encoder_outputs[sl],
            "weT": weT,
            "whT": whT,
            "hT": np.ascontiguousarray(h[sl].T),
            "bias": b_attn,
            "v": v,
        })
    return in_maps


def kernel(hidden, encoder_outputs, W_attn, b_attn, v, n_loop=1):
    from concourse.bass_utils import run_bass_kernel_spmd
    n_cores = 8
    B, S, H = encoder_outputs.shape
    nc = get_nc(B_pc=B // n_cores, S=S, H=H, n_loop=n_loop, n_cores=n_cores)
    in_maps = host_prep(hidden, encoder_outputs, W_attn, b_attn, v, n_cores)
    res = run_bass_kernel_spmd(nc, in_maps, core_ids=list(range(n_cores)))
    ctx = np.concatenate([r["ctx"] for r in res.results], axis=0)
    attn = np.concatenate([r["attn"] for r in res.results], axis=0)
    return ctx, attn
